# revision 29
# baseline (speedup 1.0000x reference)
"""Distributed Trainium2 kernel for fused multi-head attention
(QKV proj + RoPE + causal/key-padded SDPA + out-proj + bias).

Sharding: tensor-parallel over heads across 8 cores (2 heads/core, both
batches on every core).  After attention, per-(head,batch) AllToAlls
convert head-shards into sequence-row-shards so the output projection is
computed locally per row slice; the host concatenates the 8 row slices.
The AllToAlls are issued as soon as each (head, batch) finishes attention,
overlapping the next attention pair and the projection's first
accumulation half.

All matmuls run in float32r (full-rate fp32 on the PE at moving dim >= 256,
~2e-4 relative precision measured on HW).  Softmax skips max-subtraction
(scores are O(10) here, far from fp32 exp overflow); key-padding masks are
an additive -30000 activation bias (per-partition, from L), the causal mask
is a post-exp affine_select zero fill (compile-time pattern).
"""

import numpy as np

import concourse.bacc as bacc
import concourse.bass as bass
import concourse.mybir as mybir
import concourse.tile as tile
from concourse import bass_utils

B, N, D, NH = 2, 2048, 2048, 16
HD = 128               # head dim
NCORES = 8
HL = NH // NCORES      # heads per core = 2
DL = HL * HD           # local model cols = 256
NS = N // NCORES       # output row slice per core = 256
HALF = HD // 2
ET = D // HD           # 16 contraction tiles
NT = N // HD           # 16 seq tiles of 128
NCH = N // 512         # 4 free-dim chunks of 512
SCALE = 1.0 / float(np.sqrt(HD))
NEGBIG = -30000.0
ROPE_BASE = 10000.0

F32 = mybir.dt.float32
F32R = mybir.dt.float32r
I32 = mybir.dt.int32

_CACHE = {}


def build():
    if "nc" in _CACHE:
        return _CACHE["nc"]
    nc = bacc.Bacc("TRN2", target_bir_lowering=False, debug=False,
                   num_devices=NCORES)
    xT = nc.dram_tensor("xT", [B, D, N], F32R, kind="ExternalInput")
    wqkvT = nc.dram_tensor("wqkvT", [3, D, DL], F32R, kind="ExternalInput")
    wpT = nc.dram_tensor("wpT", [D, D], F32R, kind="ExternalInput")
    bp = nc.dram_tensor("bp", [1, D], F32, kind="ExternalInput")
    cosT = nc.dram_tensor("cosT", [HD, N], F32, kind="ExternalInput")
    sinT = nc.dram_tensor("sinT", [HD, N], F32, kind="ExternalInput")
    Lw = nc.dram_tensor("Lw", [1, B], I32, kind="ExternalInput")
    onesv = nc.dram_tensor("onesv", [HD, 1], F32R, kind="ExternalInput")
    out = nc.dram_tensor("out", [B, NS, D], F32, kind="ExternalOutput")

    AF = mybir.ActivationFunctionType
    ALU = mybir.AluOpType

    with tile.TileContext(nc) as tc:
        with tc.tile_pool(name="persist", bufs=1) as pp, \
             tc.tile_pool(name="dram", bufs=1, space="DRAM") as dp:
            # per-(b,h) scratch so attention can start while later batches
            # are still in the QKV phase
            qs = [[dp.tile([HD, N], F32R, name=f"qs{b}{h}")
                   for h in range(HL)] for b in range(B)]
            ks = [[dp.tile([HD, N], F32R, name=f"ks{b}{h}")
                   for h in range(HL)] for b in range(B)]
            # one AllToAll per (h, b): issued right after that pair's
            # attention completes
            ca = [[dp.tile([NCORES, HD, NS], F32R, name=f"ca{h}{b}")
                   for b in range(B)] for h in range(HL)]
            cb = [[dp.tile([NCORES, HD, NS], F32R, name=f"cb{h}{b}")
                   for b in range(B)] for h in range(HL)]

            ones = pp.tile([HD, 1], F32R)
            nc.sync.dma_start(ones[:], onesv[:])

            # key-padding additive bias per (partition=j%128, batch, jtile)
            iota = pp.tile([HD, NT], I32)
            nc.gpsimd.iota(iota[:], pattern=[[HD, NT]], base=0,
                           channel_multiplier=1)
            iotaf = pp.tile([HD, NT], F32)
            nc.vector.tensor_copy(iotaf[:], iota[:])
            lsb = pp.tile([1, B], I32)
            nc.sync.dma_start(lsb[:], Lw[:])
            lf = pp.tile([1, B], F32)
            nc.vector.tensor_copy(lf[:], lsb[:])
            lb = pp.tile([HD, B], F32)
            nc.gpsimd.partition_broadcast(lb[:], lf[:])
            kpad = pp.tile([HD, B, NT], F32)
            for b in range(B):
                nc.vector.tensor_scalar(kpad[:, b, :], iotaf[:], lb[:, b:b + 1],
                                        NEGBIG, ALU.is_ge, ALU.mult)

            # bias row broadcast to all 128 partitions
            bp1 = pp.tile([1, D], F32)
            nc.sync.dma_start(bp1[:], bp[:])
            bpb = pp.tile([HD, D], F32)
            nc.gpsimd.partition_broadcast(bpb[:], bp1[:])


            # first proj weight chunk prefetches during earlier phases;
            # V stays resident in SBUF across phases 1-2 (no DRAM roundtrip)
            pwctx = tc.tile_pool(name="projw0", bufs=1)
            pw0 = pwctx.__enter__()
            wpt0 = pw0.tile([HD, ET, 512], F32R, name="wpt0")
            nc.sync.dma_start(
                wpt0[:], wpT[:, 0:512].rearrange("(t p) f -> p t f", p=HD))
            vsctx = tc.tile_pool(name="vres", bufs=1)
            vsp = vsctx.__enter__()
            vsb = [vsp.tile([HD, NT, DL], F32R, name=f"vsb{b}")
                   for b in range(B)]

            # ---------------- Phase 1: QKV projection + RoPE ----------------
            with tc.tile_pool(name="wqkv", bufs=1) as wqp, \
                 tc.tile_pool(name="ph1x", bufs=20) as xp, \
                 tc.tile_pool(name="ph1s", bufs=3) as sp, \
                 tc.tile_pool(name="ph1p", bufs=2, space="PSUM") as pq:
                cosb = wqp.tile([HD, N], F32, tag="cosb")
                nc.sync.dma_start(cosb[:], cosT[:])
                sinb = wqp.tile([HD, N], F32, tag="sinb")
                nc.sync.dma_start(sinb[:], sinT[:])
                # weights in 4-et-group tiles (distinct tags so all stay
                # resident): first matmuls wait on ~0.6MB, not the full 6MB
                wql, wkl, wvl = [], [], []
                for g in range(4):
                    for lst, wn, i in ((wql, "wq", 0), (wkl, "wk", 1),
                                       (wvl, "wv", 2)):
                        t = wqp.tile([HD, 4, DL], F32R, tag=f"{wn}{g}",
                                     name=f"{wn}{g}")
                        nc.sync.dma_start(
                            t[:], wqkvT[i].rearrange(
                                "(t p) d -> p t d", p=HD)[:, 4 * g:4 * g + 4])
                        lst.append(t)

                for b in range(B):
                    for c4 in range(NCH):
                        nsl = slice(c4 * 512, (c4 + 1) * 512)
                        xts = []
                        for et in range(ET):
                            xt = xp.tile([HD, 512], F32R, tag="xt",
                                         name=f"xt{et}")
                            nc.sync.dma_start(
                                xt[:], xT[b, et * HD:(et + 1) * HD, nsl])
                            xts.append(xt)
                        # two 4-bank sub-iterations (one per head) so the
                        # PSUM pool double-buffers and the PE never waits
                        # for the rope/copy epilogue
                        for h in range(HL):
                            psq = pq.tile([HD, 512], F32, tag="pq")
                            psk = pq.tile([HD, 512], F32, tag="pk")
                            psv = [pq.tile([HD, DL], F32, tag=f"pv{i}",
                                           name=f"psv{i}") for i in range(2)]
                            for et in range(ET):
                                st = (et == 0)
                                en = (et == ET - 1)
                                nc.tensor.matmul(
                                    psq[:],
                                    wql[et // 4][:, et % 4,
                                                 h * HD:(h + 1) * HD],
                                    xts[et][:], start=st, stop=en)
                                nc.tensor.matmul(
                                    psk[:],
                                    wkl[et // 4][:, et % 4,
                                                 h * HD:(h + 1) * HD],
                                    xts[et][:], start=st, stop=en)
                                for i in range(2):
                                    s4 = 2 * h + i
                                    nc.tensor.matmul(
                                        psv[i][:],
                                        xts[et][:, s4 * HD:(s4 + 1) * HD],
                                        wvl[et // 4][:, et % 4, :], start=st, stop=en)
                            # copy-first RoPE: free the PSUM bank after one
                            # copy, rotate in SBUF (tables half-duplicated so
                            # every tensor_tensor has equal partition bases)
                            for src, dst in ((psq, qs[b][h]),
                                             (psk, ks[b][h])):
                                stg = sp.tile([HD, 512], F32R, tag="stg")
                                t12 = sp.tile([HD, 512], F32, tag="t12")
                                nc.vector.tensor_copy(stg[:], src[:])
                                nc.vector.tensor_mul(
                                    t12[:HALF, :], stg[HALF:, :],
                                    sinb[HALF:, nsl])
                                nc.vector.tensor_mul(
                                    t12[HALF:, :], stg[:HALF, :],
                                    sinb[:HALF, nsl])
                                nc.vector.tensor_mul(stg[:], stg[:],
                                                     cosb[:, nsl])
                                nc.vector.tensor_sub(stg[:HALF, :],
                                                     stg[:HALF, :],
                                                     t12[:HALF, :])
                                nc.vector.tensor_add(stg[HALF:, :],
                                                     stg[HALF:, :],
                                                     t12[HALF:, :])
                                nc.sync.dma_start(dst[:, nsl], stg[:])
                            for i in range(2):
                                nc.vector.tensor_copy(
                                    vsb[b][:, c4 * 4 + 2 * h + i, :],
                                    psv[i][:])

            # ------------- Phase 2: attention per (h, b) + AllToAll --------
            with tc.tile_pool(name="att", bufs=2) as ap_, \
                 tc.tile_pool(name="atts", bufs=6) as sp2, \
                 tc.tile_pool(name="attp", bufs=4, space="PSUM") as pq2, \
                 tc.tile_pool(name="attpo", bufs=2, space="PSUM") as pq3:
                for h in range(HL):
                    for b in range(B):
                        qt = ap_.tile([HD, N], F32R, tag="qt")
                        nc.sync.dma_start(qt[:], qs[b][h][:])
                        kt = ap_.tile([HD, N], F32R, tag="kt")
                        nc.sync.dma_start(kt[:], ks[b][h][:])
                        vt = vsb[b][:, :, h * HD:(h + 1) * HD]
                        for c4 in range(NCH):
                            nsl = slice(c4 * 512, (c4 + 1) * 512)
                            pso = pq3.tile([HD, 512], F32, tag="pso")
                            psd = pq3.tile([1, 512], F32, tag="psd")
                            njt = 4 * c4 + 4
                            for jt in range(njt):
                                # diagonal tiles (jt = 4*c4+r, r>0) have no
                                # valid columns below 128*r: stream only the
                                # live range through PE/ACT/DVE
                                r = jt - 4 * c4
                                lo = 128 * r if r > 0 else 0
                                w = 512 - lo
                                pss = pq2.tile([HD, 512], F32, tag="pss")
                                nc.tensor.matmul(
                                    pss[:, lo:], kt[:, jt * HD:(jt + 1) * HD],
                                    qt[:, c4 * 512 + lo:(c4 + 1) * 512],
                                    start=True, stop=True)
                                pt = sp2.tile([HD, 512], F32R, tag="pt")
                                nc.scalar.activation(
                                    pt[:, lo:], pss[:, lo:], AF.Exp,
                                    bias=kpad[:, b, jt:jt + 1], scale=SCALE)
                                if r >= 0 and jt >= 4 * c4:
                                    nc.gpsimd.affine_select(
                                        out=pt[:, lo:], in_=pt[:, lo:],
                                        compare_op=ALU.is_ge, fill=0.0,
                                        base=0,
                                        pattern=[[1, w]],
                                        channel_multiplier=-1)
                                nc.tensor.matmul(pso[:, lo:], vt[:, jt, :],
                                                 pt[:, lo:],
                                                 start=(jt == 0),
                                                 stop=(jt == njt - 1))
                                nc.tensor.matmul(psd[:, lo:], ones[:],
                                                 pt[:, lo:],
                                                 start=(jt == 0),
                                                 stop=(jt == njt - 1))
                            rec = sp2.tile([1, 512], F32, tag="rec")
                            nc.vector.reciprocal(rec[:], psd[:])
                            rb = sp2.tile([HD, 512], F32, tag="rb")
                            nc.gpsimd.partition_broadcast(rb[:], rec[:])
                            ou = sp2.tile([HD, 512], F32R, tag="ou")
                            nc.vector.tensor_mul(ou[:], pso[:], rb[:])
                            for r in range(2):
                                nc.sync.dma_start(
                                    ca[h][b][2 * c4 + r],
                                    ou[:, r * NS:(r + 1) * NS])
                        # exchange this (head, batch) while the next pair
                        # computes
                        nc.gpsimd.collective_compute(
                            "AllToAll", mybir.AluOpType.bypass,
                            replica_groups=[list(range(NCORES))],
                            ins=[ca[h][b].opt()], outs=[cb[h][b].opt()])

            vsctx.__exit__(None, None, None)

            # ---------------- Phase 3: output projection ----------------
            with tc.tile_pool(name="proj", bufs=1) as pj, \
                 tc.tile_pool(name="projw", bufs=3) as pw, \
                 tc.tile_pool(name="projs", bufs=4) as po, \
                 tc.tile_pool(name="projp", bufs=4, space="PSUM") as pq4:
                # asb[h][p, b, s, n] = cb[h][b][s, p, n]
                asb = [pj.tile([HD, B, NCORES, NS], F32R, name=f"asb{h}")
                       for h in range(HL)]
                for h in range(HL):
                    for b in range(B):
                        for s in range(NCORES):
                            nc.sync.dma_start(asb[h][:, b, s], cb[h][b][s])
                for f4 in range(NCH):
                    fsl = slice(f4 * 512, (f4 + 1) * 512)
                    if f4 == 0:
                        wpt = wpt0
                    else:
                        wpt = pw.tile([HD, ET, 512], F32R, tag="wpt")
                        nc.sync.dma_start(
                            wpt[:],
                            wpT[:, fsl].rearrange("(t p) f -> p t f", p=HD))
                    for b in range(B):
                        for m in range(NS // HD):
                            psp = pq4.tile([HD, 512], F32, tag="psp")
                            # h=0 tiles first: they are exchanged earlier, so
                            # accumulation can begin while h=1 is in flight
                            for gi in range(ET):
                                h, s = gi // NCORES, gi % NCORES
                                nc.tensor.matmul(
                                    psp[:],
                                    asb[h][:, b, s, m * HD:(m + 1) * HD],
                                    wpt[:, 2 * s + h, :],
                                    start=(gi == 0), stop=(gi == ET - 1))
                            ot = po.tile([HD, 512], F32, tag="ot")
                            nc.vector.tensor_add(ot[:], psp[:], bpb[:, fsl])
                            nc.sync.dma_start(
                                out[b, m * HD:(m + 1) * HD, fsl], ot[:])
            pwctx.__exit__(None, None, None)

    nc.compile()
    _CACHE["nc"] = nc
    return nc


def _prep_inputs(x, Wqkv, Wproj, bproj, L):
    x = np.asarray(x, np.float32)
    Wqkv = np.asarray(Wqkv, np.float32)
    Wproj = np.asarray(Wproj, np.float32)
    bproj = np.asarray(bproj, np.float32)
    L = np.asarray(L, np.int32)

    xT = np.ascontiguousarray(x.transpose(0, 2, 1))
    wpT = np.ascontiguousarray(Wproj.T)
    inv = 1.0 / (ROPE_BASE ** (np.arange(0, HD, 2, dtype=np.float32) / HD))
    ang = np.arange(N, dtype=np.float32)[:, None] * inv[None, :]
    cos1 = np.cos(ang).T.astype(np.float32)          # [64, N]
    sin1 = np.sin(ang).T.astype(np.float32)
    cosT = np.ascontiguousarray(np.vstack([cos1, cos1]))   # [128, N]
    sinT = np.ascontiguousarray(np.vstack([sin1, sin1]))
    Lw = L.reshape(1, B).astype(np.int32)
    bp = bproj.reshape(1, D)

    in_maps = []
    for c in range(NCORES):
        sl = slice(c * DL, (c + 1) * DL)
        w3 = np.stack([
            np.ascontiguousarray(Wqkv[0 * D:1 * D][sl].T),
            np.ascontiguousarray(Wqkv[1 * D:2 * D][sl].T),
            np.ascontiguousarray(Wqkv[2 * D:3 * D][sl].T),
        ])
        in_maps.append({
            "xT": xT, "wqkvT": w3, "wpT": wpT, "bp": bp,
            "cosT": cosT, "sinT": sinT, "Lw": Lw,
            "onesv": np.ones((HD, 1), np.float32),
        })
    return in_maps


def run(x, Wqkv, Wproj, bproj, L, trace=False, tmpdir=None):
    nc = build()
    in_maps = _prep_inputs(x, Wqkv, Wproj, bproj, L)
    kw = {}
    if tmpdir is not None:
        kw["tmpdir"] = tmpdir
    res = bass_utils.run_bass_kernel_spmd(
        nc, in_maps, core_ids=list(range(NCORES)), trace=trace, **kw)
    full = np.empty((B, N, D), np.float32)
    for c in range(NCORES):
        full[:, c * NS:(c + 1) * NS, :] = res.results[c]["out"]
    return full, res


def kernel(x, Wqkv, Wproj, bproj, L, n_heads):
    assert int(n_heads) == NH
    full, _ = run(x, Wqkv, Wproj, bproj, L, trace=False)
    return full
